# revision 4
# baseline (speedup 1.0000x reference)
"""Trainium2 Bass kernel for GQA attention (B=1, S=2048, D=2048, H=16, KVH=4, HD=128).

Instruction-count-optimized revision of the tensor-parallel baseline.
On this environment, execution time is dominated by a ~50-90us
per-instruction service cost (globally serialized across engines), so
the kernel is structured to minimize the total instruction count:

  - KV dedup: core c computes Q heads {2c, 2c+1} plus ONE of k/v for kv
    head c//2 (even cores: k, odd cores: v) -> 192 projection matmuls
    instead of 256. The pair exchanges k/v via a 2-rank AllGather.
  - x is loaded in 4 big DMAs, constants in 3.
  - PSUM groups sized to write wide contiguous regions so each
    PSUM->SBUF move is one wide copy, issued on the scalar (ACT)
    engine to keep the vector engine off the critical path.
  - Softmax reciprocals deferred to one [1, 4096] op for both heads,
    one DRAM-bounce broadcast, one big normalize multiply.
  - Output projection accumulates into one [2048, 2048] DRAM buffer and
    uses a SINGLE ReduceScatter (fixed collective cost ~465us dominates;
    one 16MB RS is cheaper than four 4MB ones).
  - Head-dim permutation trick retained: wq/wk columns permuted per head
    to [even dims, odd dims] so RoPE is 6 wide DVE ops; q/k permuted
    identically => scores unchanged; v untouched.
"""

import numpy as np
from contextlib import ExitStack

import concourse.bacc as bacc
import concourse.tile as tile
import concourse.mybir as mybir
from concourse.bass_utils import run_bass_kernel_spmd

S = 2048
D = 2048
H = 16
KVH = 4
HD = 128
NCORES = 8
F32 = mybir.dt.float32
SCALE = float(1.0 / np.sqrt(HD))
NEG = -1e9

MM_DTYPE = F32

_BUILD_CACHE = {}


def _mm(nc, out, lhsT, rhs, start, stop, dtype):
    if dtype != F32:
        lhsT = lhsT.bitcast(dtype)
        rhs = rhs.bitcast(dtype)
    nc.tensor.matmul(out, lhsT, rhs, start=start, stop=stop)


def _emit_body(nc, tc, io, mm_dtype, bodyidx):
    with ExitStack() as ctx:
        # ---------------- constants ----------------
        consts = ctx.enter_context(tc.tile_pool(name="consts", bufs=1))
        wo_sb = consts.tile([128, 2, 2048], F32, tag="wo")
        nc.sync.dma_start(out=wo_sb[:], in_=io["wo"].rearrange("(h p) n -> p h n", p=128))
        mi_sb = consts.tile([128, 2176], F32, tag="mi")  # mask[2048] | ident[128]
        nc.sync.dma_start(out=mi_sb[:], in_=io["mi"][:])
        cossin_sb = consts.tile([64, 2, 2048], F32, tag="cossin")
        nc.sync.dma_start(out=cossin_sb[:], in_=io["cossin"][:])
        ones_sb = consts.tile([128, 1], F32, tag="ones")
        nc.vector.memset(ones_sb[:], 1.0)

        # persistent activations
        acts = ctx.enter_context(tc.tile_pool(name="acts", bufs=1))
        qkT = acts.tile([128, 3, 2048], F32, tag="qk")  # [.,0,.]=q0 [.,1,.]=q1 [.,2,.]=k
        vT_sb = acts.tile([128, 2048], F32, tag="vT")
        v_sb = acts.tile([128, 16, 128], F32, tag="v")
        attn = acts.tile([128, 2, 2048], F32, tag="attn")
        kvst = acts.tile([128, 2048], F32, tag="kvst")  # own k-or-v (pre-exchange)
        sums = acts.tile([1, 2, 2048], F32, tag="sums")

        dram = ctx.enter_context(tc.tile_pool(name="dram", bufs=1, space="DRAM"))

        # ---------------- phase 1: QKV projections (transposed) ----------------
        xview = io["xT"].rearrange("(t p) s -> p t s", p=128)
        with tc.tile_pool(name="p1sb", bufs=1) as p1sb, \
             tc.tile_pool(name="p1ps", bufs=1, space="PSUM") as p1ps:
            wqkv_sb = p1sb.tile([128, 16, 384], F32, tag="wqkv")
            nc.sync.dma_start(out=wqkv_sb[:],
                              in_=io["wqkv"].rearrange("(t p) n -> p t n", p=128))
            for sh in range(4):
                xh = p1sb.tile([128, 16, 512], F32, tag="xh")
                nc.sync.dma_start(out=xh[:], in_=xview[:, :, sh * 512:(sh + 1) * 512])
                ps_qq = p1ps.tile([128, 2, 512], F32, tag="psqq")
                ps_kv = p1ps.tile([128, 512], F32, tag="pskv")
                for t in range(16):
                    st = t == 0
                    sp = t == 15
                    _mm(nc, ps_qq[:, 0, :], wqkv_sb[:, t, 0:128],
                        xh[:, t, :], st, sp, mm_dtype)
                    _mm(nc, ps_qq[:, 1, :], wqkv_sb[:, t, 128:256],
                        xh[:, t, :], st, sp, mm_dtype)
                    _mm(nc, ps_kv[:, :], wqkv_sb[:, t, 256:384],
                        xh[:, t, :], st, sp, mm_dtype)
                ssl = slice(sh * 512, (sh + 1) * 512)
                nc.scalar.copy(qkT[:, 0:2, ssl], ps_qq[:])
                nc.scalar.copy(kvst[:, ssl], ps_kv[:])

        # ---------------- phase 1.2: pair AllGather k/v ----------------
        # even core holds kT, odd holds vT; AG slot0 = k, slot1 = v for both.
        kvout = dram.tile([128, 2048], F32, name=f"kvout_{bodyidx}")
        nc.sync.dma_start(out=kvout[:], in_=kvst[:])
        kvall = dram.tile([256, 2048], F32, name=f"kvall_{bodyidx}")
        nc.gpsimd.collective_compute(
            "AllGather", mybir.AluOpType.bypass,
            replica_groups=[[0, 1], [2, 3], [4, 5], [6, 7]],
            ins=[kvout.opt()], outs=[kvall.opt()])
        nc.sync.dma_start(out=qkT[:, 2, :], in_=kvall[0:128, :])
        nc.sync.dma_start(out=vT_sb[:], in_=kvall[128:256, :])

        # ---------------- phase 1.5: RoPE on q0, q1, k (in place) ----------------
        with tc.tile_pool(name="rope", bufs=1) as rp:
            cos_b = cossin_sb[:, 0, :].unsqueeze(1).broadcast_to((64, 3, 2048))
            sin_b = cossin_sb[:, 1, :].unsqueeze(1).broadcast_to((64, 3, 2048))
            t0 = qkT[0:64, :, :]
            t1c = rp.tile([64, 3, 2048], F32, tag="t1c")
            nc.sync.dma_start(out=t1c[:], in_=qkT[64:128, :, :])
            o1 = rp.tile([64, 3, 2048], F32, tag="o1")
            tmp = rp.tile([64, 3, 2048], F32, tag="tmp")
            nc.vector.tensor_mul(o1[:], t0, sin_b)
            nc.vector.tensor_mul(tmp[:], t1c[:], cos_b)
            nc.vector.tensor_add(o1[:], o1[:], tmp[:])
            nc.vector.tensor_mul(tmp[:], t1c[:], sin_b)
            nc.vector.tensor_mul(t1c[:], t0, cos_b)
            nc.vector.tensor_sub(qkT[0:64, :, :], t1c[:], tmp[:])
            nc.sync.dma_start(out=qkT[64:128, :, :], in_=o1[:])

        # ---------------- phase 1.3 + 2: transposes and attention ----------------
        ident = mi_sb[:, 2048:2176]
        mi4 = mi_sb[:, 0:2048].rearrange("p (a b) -> p a b", a=4)
        with tc.tile_pool(name="atsb", bufs=1) as atsb, \
             tc.tile_pool(name="atps", bufs=1, space="PSUM") as atps:
            for j4 in range(4):
                tp4 = atps.tile([128, 4, 128], F32, tag="tp4")
                for i in range(4):
                    j = 4 * j4 + i
                    nc.tensor.transpose(tp4[:, i, :],
                                        vT_sb[:, j * 128:(j + 1) * 128], ident)
                nc.scalar.copy(v_sb[:, 4 * j4:4 * j4 + 4, :], tp4[:])

            for h in range(2):
                for qc in range(4):
                    qsl = slice(qc * 512, (qc + 1) * 512)
                    ps_o = atps.tile([128, 512], F32, tag="ps_o")
                    pt = atsb.tile([128, 16, 512], F32, tag="pt")
                    nq = qc + 1
                    for g in range(nq):
                        j0 = 4 * g
                        quad = atps.tile([128, 4, 512], F32, tag="quad")
                        for i in range(4):
                            _mm(nc, quad[:, i, :],
                                qkT[:, 2, (j0 + i) * 128:(j0 + i + 1) * 128],
                                qkT[:, h, qsl], True, True, mm_dtype)
                        if g == nq - 1:
                            nc.vector.tensor_add(quad[:], quad[:], mi4)
                        nc.scalar.activation(pt[:, j0:j0 + 4, :], quad[:],
                                             mybir.ActivationFunctionType.Exp,
                                             scale=SCALE)
                        for i in range(4):
                            _mm(nc, ps_o[:], v_sb[:, j0 + i, :], pt[:, j0 + i, :],
                                j0 + i == 0, j0 + i == 4 * nq - 1, mm_dtype)
                    nc.scalar.copy(attn[:, h, qsl], ps_o[:])
                    njt = 4 * nq
                    red = atsb.tile([128, 512], F32, tag="red")
                    nc.vector.tensor_reduce(
                        red[:], pt[:, 0:njt, :].rearrange("p a b -> p b a"),
                        axis=mybir.AxisListType.X, op=mybir.AluOpType.add)
                    ps_cs = atps.tile([1, 512], F32, tag="ps_cs")
                    _mm(nc, ps_cs[:], ones_sb[:], red[:], True, True, mm_dtype)
                    nc.vector.tensor_copy(sums[0:1, h, qsl], ps_cs[:])

            # normalize both heads at once via a DRAM-bounce broadcast
            recip = atsb.tile([1, 4096], F32, tag="recip")
            nc.vector.reciprocal(recip[:], sums[:].rearrange("p a b -> p (a b)"))
            rb = dram.tile([1, 4096], F32, name=f"rb_{bodyidx}")
            nc.sync.dma_start(out=rb[:], in_=recip[:])
            bc = atsb.tile([128, 4096], F32, tag="bc")
            nc.sync.dma_start(out=bc[:], in_=rb.to_broadcast((128, 4096)))
            nc.vector.tensor_mul(attn[:].rearrange("p a b -> p (a b)"),
                                 attn[:].rearrange("p a b -> p (a b)"), bc[:])

        # ---------------- phase 3: output projection + single ReduceScatter ----------------
        woacc = dram.tile([2048, 2048], F32, name=f"woacc_{bodyidx}")
        woacc_v = woacc.rearrange("(a p) n -> p a n", p=128)  # [128, 16, 2048]
        with tc.tile_pool(name="wosb", bufs=1) as wosb, \
             tc.tile_pool(name="wops", bufs=1, space="PSUM") as wops:
            for q4 in range(4):
                o_sb = wosb.tile([128, 4, 2048], F32, tag="osb")
                for tg in range(2):  # 2 s-tiles per 8-bank PSUM group
                    ps_wo = wops.tile([128, 2, 2048], F32, tag="pswo")
                    for sti in range(2):
                        st = 4 * q4 + 2 * tg + sti
                        ssl = slice(st * 128, (st + 1) * 128)
                        for n in range(4):
                            nsl = slice(n * 512, (n + 1) * 512)
                            _mm(nc, ps_wo[:, sti, nsl], attn[:, 0, ssl],
                                wo_sb[:, 0, nsl], True, False, mm_dtype)
                            _mm(nc, ps_wo[:, sti, nsl], attn[:, 1, ssl],
                                wo_sb[:, 1, nsl], False, True, mm_dtype)
                    nc.scalar.copy(o_sb[:, 2 * tg:2 * tg + 2, :], ps_wo[:])
                nc.sync.dma_start(out=woacc_v[:, 4 * q4:4 * q4 + 4, :], in_=o_sb[:])

        rs_out = dram.tile([256, 2048], F32, name=f"rsout_{bodyidx}")
        nc.gpsimd.collective_compute(
            "ReduceScatter", mybir.AluOpType.add,
            replica_groups=[list(range(NCORES))],
            ins=[woacc.opt()], outs=[rs_out.opt()])
        nc.sync.dma_start(out=io["out"][:], in_=rs_out[:])


def build(mm_dtype=None, repeat=1, num_devices=NCORES):
    mm_dtype = mm_dtype or MM_DTYPE
    key = (str(mm_dtype), repeat, num_devices)
    if key in _BUILD_CACHE:
        return _BUILD_CACHE[key]
    nc = bacc.Bacc("TRN2", target_bir_lowering=False, debug=False,
                   num_devices=num_devices)
    io = {
        "xT": nc.dram_tensor("xT", [D, S], F32, kind="ExternalInput").ap(),
        "wqkv": nc.dram_tensor("wqkv", [D, 384], F32, kind="ExternalInput").ap(),
        "wo": nc.dram_tensor("wo", [256, D], F32, kind="ExternalInput").ap(),
        "cossin": nc.dram_tensor("cossin", [64, 2, 2048], F32, kind="ExternalInput").ap(),
        "mi": nc.dram_tensor("mi", [128, 2176], F32, kind="ExternalInput").ap(),
        "out": nc.dram_tensor("out", [S // NCORES, D], F32, kind="ExternalOutput").ap(),
    }
    with tile.TileContext(nc) as tc:
        for r in range(repeat):
            _emit_body(nc, tc, io, mm_dtype, r)
    nc.compile()
    _BUILD_CACHE[key] = nc
    return nc


def prepare_in_maps(x, wq, wk, wv, wo, freqs_cos, freqs_sin):
    x2d = np.asarray(x, dtype=np.float32).reshape(S, D)
    xT = np.ascontiguousarray(x2d.T)
    cos = np.asarray(freqs_cos, np.float32).T  # [64, 2048]
    sin = np.asarray(freqs_sin, np.float32).T
    cossin = np.ascontiguousarray(np.stack([cos, sin], axis=1))  # [64, 2, 2048]

    # even dims first, then odd dims (applied to q and k only)
    perm = np.concatenate([np.arange(0, HD, 2), np.arange(1, HD, 2)])

    # boundary mask tiles in scoresT layout: visible iff jl <= ql - 128*r
    jl = np.arange(128)[:, None]
    ql = np.arange(512)[None, :]
    maskt = np.empty((128, 4, 512), np.float32)
    for r in range(4):
        maskt[:, r, :] = np.where(jl <= ql - 128 * r, 0.0, NEG)
    ident = np.eye(128, dtype=np.float32)
    mi = np.ascontiguousarray(
        np.concatenate([maskt.reshape(128, 2048), ident], axis=1))  # [128, 2176]

    wq = np.asarray(wq, np.float32)
    wk = np.asarray(wk, np.float32)
    wv = np.asarray(wv, np.float32)
    wo = np.asarray(wo, np.float32)

    in_maps = []
    for c in range(NCORES):
        g = c // 2
        wq_c = wq[:, 2 * c * HD:(2 * c + 2) * HD].reshape(D, 2, HD)[:, :, perm].reshape(D, 2 * HD)
        if c % 2 == 0:
            kv_c = wk[:, g * HD:(g + 1) * HD][:, perm]
        else:
            kv_c = wv[:, g * HD:(g + 1) * HD]
        wqkv_c = np.ascontiguousarray(np.concatenate([wq_c, kv_c], axis=1))  # [D, 384]
        wo_c = np.ascontiguousarray(wo[2 * c * HD:(2 * c + 2) * HD, :])  # [256, D]
        in_maps.append({
            "xT": xT,
            "wqkv": wqkv_c,
            "wo": wo_c,
            "cossin": cossin,
            "mi": mi,
        })
    return in_maps


def assemble_output(results):
    # single RS: core c holds output rows [256c, 256c+256)
    return np.ascontiguousarray(
        np.concatenate([results[c]["out"] for c in range(NCORES)], axis=0)
    ).reshape(1, S, D)


def kernel(x, wq, wk, wv, wo, freqs_cos, freqs_sin, mask):
    nc = build()
    in_maps = prepare_in_maps(x, wq, wk, wv, wo, freqs_cos, freqs_sin)
    res = run_bass_kernel_spmd(nc, in_maps, core_ids=list(range(NCORES)))
    return assemble_output(res.results).astype(np.float32)
